# revision 8
# baseline (speedup 1.0000x reference)
"""Multi-head attention (b=2, p=16, n=512, d=512, h=8, dh=64) on 8 TRN2 cores.

Data-parallel over the 32 (b,p) sequences: 4 sequences per core, no
collectives.  Per-core dataflow (everything "T" = feature-on-partition):

  xT  (d,n)  --W_qkv stationary-->  qT,kT (e,n)   [e-tile = 2 heads]
  xT chunks stationary, W_v moving ->  v natural (n, h*dh)
  dots: per head pair, K=64 row-tiled (A rows 0:64 / B rows 64:128) so the
    two heads' score matmuls run CONCURRENTLY in the PE array; both heads'
    4 score banks are evacuated by ONE ScalarE exp (FD=2048), so the banks
    free atomically and the next dots pair issues back-to-back.
  attnv: per (head, key-tile), M=64 col-tiled (A cols 0:64 / B cols
    64:128) into one PSUM bank -> concurrent pair, accumulated over key
    tiles with per-element has_written semantics (single bank-clear).
  softmax denominators: tiny ones-matmuls (M=32, 4x col-tiled across the
    array) accumulate per-(head,key-tile) partials into a zeroed PSUM
    bank; one reduce matmul folds partials to per-head totals at rows
    32t/32t+1; reciprocal = ScalarE ln then exp(-x) (same table set as
    the softmax exp); R = 4x row-tiled K=2 broadcast matmuls; oT *= R.
  yT = W_out.T @ oT + b  (bf16 out, DMA to DRAM; host widens to f32)

QKV work for sequence s+1 is interleaved into the attention pairs of
sequence s so the TensorEngine never starves while ScalarE runs exp.
Host side transposes x into xT and the returned yT back to y.
"""

import os
import sys

import numpy as np

for _p in ("/opt/trn_rl_repo", "/root/.axon_site/_ro/trn_rl_repo"):
    if os.path.isdir(_p) and _p not in sys.path:
        sys.path.insert(0, _p)

import concourse.bass as bass  # noqa: E402
import concourse.mybir as mybir  # noqa: E402
from concourse import bacc  # noqa: E402
from concourse.tile import TileContext  # noqa: E402

F32 = mybir.dt.float32
BF16 = mybir.dt.bfloat16

N_CORES = 8
SEQ_PER_CORE = 4  # (b*p)=32 sequences / 8 cores
N = 512  # tokens per sequence
D = 512  # model dim
HEADS = 8
DH = 64
SCALE = DH**-0.5
NT = N // 128  # 4 token tiles
DT = D // 128  # 4 dim tiles

EXP_F = mybir.ActivationFunctionType.Exp
LOG_F = mybir.ActivationFunctionType.Ln
MULT = mybir.AluOpType.mult


def _pin_act_tables():
    """Make Exp and Ln resolve to the one table set that holds BOTH
    (natural_log_exp_and_others), so ScalarE never swaps table sets
    between the softmax exp and the reciprocal ln.  Set order (and thus
    act_func_set_id) is preserved; only the function->set resolution
    changes."""
    import concourse.hw_specs as hw_specs

    if getattr(bacc, "_act_tables_pinned", False):
        return
    real = hw_specs.get_activation_tables

    def patched(module_arch):
        tables = real(module_arch)
        both = {
            name
            for name, s in tables.items()
            if EXP_F in s and LOG_F in s
        }
        if not both:
            return tables
        out = {}
        for name, s in tables.items():
            if name not in both:
                s = s - {EXP_F, LOG_F}
            out[name] = s
        return out

    bacc.get_activation_tables = patched
    bacc._act_tables_pinned = True


def build_nc():
    """Build the per-core SPMD Bass program (same program on all 8 cores)."""
    _pin_act_tables()
    nc = bacc.Bacc("TRN2", target_bir_lowering=False)

    xT = nc.declare_dram_parameter(
        "xT", [SEQ_PER_CORE, DT, 128, N], BF16, isOutput=False
    )
    wqkv = nc.declare_dram_parameter("wqkv", [DT, 128, 3 * D], BF16, isOutput=False)
    wout = nc.declare_dram_parameter("wout", [DT, 128, D], BF16, isOutput=False)
    bout = nc.declare_dram_parameter("bout", [D], F32, isOutput=False)
    p2d = nc.declare_dram_parameter("p2d", [128, 128], BF16, isOutput=False)
    onesd = nc.declare_dram_parameter("onesd", [128, 8, 32], BF16, isOutput=False)
    prd = nc.declare_dram_parameter("prd", [128, 98], BF16, isOutput=False)
    out = nc.declare_dram_parameter(
        "out", [SEQ_PER_CORE, DT, 128, N], BF16, isOutput=True
    )

    with TileContext(nc) as tc:
        with (
            tc.tile_pool(name="consts", bufs=1) as cpool,
            tc.tile_pool(name="xin", bufs=2) as xpool,
            tc.tile_pool(name="qk", bufs=2) as qkpool,
            tc.tile_pool(name="vt", bufs=2) as vpool,
            tc.tile_pool(name="expt", bufs=2) as epool,
            tc.tile_pool(name="ot", bufs=2) as otpool,
            tc.tile_pool(name="small", bufs=2) as spool,
            tc.tile_pool(name="yout", bufs=2) as ypool,
            tc.tile_pool(name="dab", bufs=1, space="PSUM") as dabpool,
            tc.tile_pool(name="ops", bufs=1, space="PSUM") as opspool,
            tc.tile_pool(name="sums", bufs=1, space="PSUM") as smpool,
            tc.tile_pool(name="psq", bufs=2, space="PSUM") as psq,
        ):
            # ---- constants ------------------------------------------------
            wq_sb = cpool.tile([128, DT, 3 * D], BF16, tag="wq")
            for e in range(12):
                nc.sync.dma_start(
                    wq_sb[:, :, e * 128 : (e + 1) * 128],
                    wqkv.rearrange("t p e -> p t e")[:, :, e * 128 : (e + 1) * 128],
                )
            wo_sb = cpool.tile([128, DT, D], BF16, tag="wo")
            nc.sync.dma_start(wo_sb[:], wout.rearrange("t p e -> p t e"))
            b_sb = cpool.tile([128, DT], F32, tag="b")
            nc.sync.dma_start(b_sb[:], bout.rearrange("(t p) -> p t", p=128))
            p2_sb = cpool.tile([128, 128], BF16, tag="p2")
            nc.sync.dma_start(p2_sb[:], p2d[:])
            ones_sb = cpool.tile([128, 8, 32], BF16, tag="ones")
            nc.sync.dma_start(ones_sb[:], onesd[:])
            pr_sb = cpool.tile([128, 98], BF16, tag="pr")
            nc.sync.dma_start(pr_sb[:], prd[:])

            seq_tiles = {}

            def qkv_alloc(s):
                """Allocate per-seq tiles + start the xT DMA."""
                xt = xpool.tile([128, DT, N], BF16, tag="x")
                for dt in range(DT):
                    nc.sync.dma_start(
                        xt[:, dt, :], xT[s].rearrange("t p n -> p t n")[:, dt, :]
                    )
                q_sb = qkpool.tile([128, DT, N], BF16, tag="q")
                k_sb = qkpool.tile([128, DT, N], BF16, tag="k")
                vt = vpool.tile([128, NT, HEADS * DH], BF16, tag="v")
                seq_tiles[s] = (xt, q_sb, k_sb, vt)

            def qkv_etile(s, et):
                """One QKV output tile (et 0..7 = qT/kT e-tiles, 8..11 = v
                natural n-tiles): 4 accumulating matmuls + evacuation."""
                xt, q_sb, k_sb, vt = seq_tiles[s]
                ps = psq.tile([128, 512], F32, tag="ps")
                if et < 8:
                    for dt in range(DT):
                        nc.tensor.matmul(
                            ps[:],
                            lhsT=wq_sb[:, dt, et * 128 : (et + 1) * 128],
                            rhs=xt[:, dt, :],
                            start=(dt == 0),
                            stop=(dt == DT - 1),
                        )
                    dest = q_sb[:, et, :] if et < 4 else k_sb[:, et - 4, :]
                    nc.vector.tensor_copy(dest, ps[:])
                else:
                    nt = et - 8
                    for dt in range(DT):
                        nc.tensor.matmul(
                            ps[:],
                            lhsT=xt[:, dt, nt * 128 : (nt + 1) * 128],
                            rhs=wq_sb[:, dt, 2 * D : 3 * D],
                            start=(dt == 0),
                            stop=(dt == DT - 1),
                        )
                    nc.vector.tensor_copy(vt[:, nt, :], ps[:])

            # ---- prologue: QKV for sequence 0 -----------------------------
            qkv_alloc(0)
            for et in range(12):
                qkv_etile(0, et)

            for s in range(SEQ_PER_CORE):
                _, q_sb, k_sb, vt = seq_tiles[s]
                oT = otpool.tile([128, DT, N], BF16, tag="o")
                sums_ps = smpool.tile([128, N], F32, tag="sums")
                if s + 1 < SEQ_PER_CORE:
                    qkv_alloc(s + 1)

                fill = []
                if s + 1 < SEQ_PER_CORE:
                    fill += [(lambda s_=s + 1, e_=e: qkv_etile(s_, e_))
                             for e in range(12)]
                fi = 0

                def filler(k):
                    nonlocal fi
                    for _ in range(k):
                        if fi < len(fill):
                            fill[fi]()
                            fi += 1

                exp_tiles = {}

                def dots_half(t, jh):
                    """Scores for head pair t, key tiles 2jh/2jh+1: 4 row-
                    tiled matmuls (A/B concurrent) + ONE exp over all 4
                    banks so they free atomically."""
                    dab = dabpool.tile([128, NT, N], F32, tag="dab")
                    for jj in range(2):
                        jt = 2 * jh + jj
                        nc.tensor.matmul(
                            dab[:, jj, :],
                            lhsT=k_sb[0:64, t, jt * 128 : (jt + 1) * 128],
                            rhs=q_sb[0:64, t, :],
                            start=True,
                            stop=True,
                        )
                        nc.tensor.matmul(
                            dab[:, 2 + jj, :],
                            lhsT=k_sb[64:128, t, jt * 128 : (jt + 1) * 128],
                            rhs=q_sb[64:128, t, :],
                            start=True,
                            stop=True,
                            tile_position=(64, 0),
                        )
                    e = exp_tiles[t]
                    nc.scalar.activation(
                        e[:, jh, :, :], dab[:, :, :], EXP_F, scale=SCALE
                    )

                def attnv_half(t, jh, o_ps):
                    """attn @ v for key tiles 2jh/2jh+1: col-tiled M=64
                    pairs (A cols 0:64, B cols 64:128) into one bank."""
                    e = exp_tiles[t]
                    for jj in range(2):
                        jt = 2 * jh + jj
                        nc.tensor.matmul(
                            o_ps[0:64, :],
                            lhsT=vt[:, jt, (2 * t) * DH : (2 * t + 1) * DH],
                            rhs=e[:, jh, jj, :],
                            start=(jt == 0),
                            stop=(jt == NT - 1),
                            skip_group_check=True,
                        )
                        nc.tensor.matmul(
                            o_ps[64:128, :],
                            lhsT=vt[:, jt, (2 * t + 1) * DH : (2 * t + 2) * DH],
                            rhs=e[:, jh, 2 + jj, :],
                            start=(jt == 0),
                            stop=(jt == NT - 1),
                            skip_group_check=True,
                            tile_position=(0, 64),
                        )

                def sums_half(t, jh):
                    """Denominator partials: 4 ones-matmuls in 4 distinct
                    column groups (concurrent).  Partial for head 2t+p over
                    key tile 2jh+jj lands at row 32*(2jj+p) + 4t + 2jh."""
                    e = exp_tiles[t]
                    for jj in range(2):
                        for p in range(2):
                            g = 2 * jj + p
                            nc.tensor.matmul(
                                sums_ps[32 * g : 32 * g + 32, :],
                                lhsT=ones_sb[:, 2 * t + jh, :],
                                rhs=e[:, jh, (2 + jj) if p else jj, :],
                                start=(t == 0 and jh == 0),
                                stop=True,
                                skip_group_check=True,
                                tile_position=(0, 32 * g),
                            )

                for t in range(4):
                    exp_tiles[t] = epool.tile(
                        [128, 2, NT, N], BF16, tag="exp", name=f"exp{s}_{t}"
                    )
                    o_ps = opspool.tile([128, N], F32, tag="o")
                    dots_half(t, 0)
                    filler(1)
                    dots_half(t, 1)
                    attnv_half(t, 0, o_ps)
                    sums_half(t, 0)
                    filler(1)
                    attnv_half(t, 1, o_ps)
                    sums_half(t, 1)
                    nc.vector.tensor_copy(oT[:, t, :], o_ps[:])
                    filler(1)

                # ---- softmax denominators -> R, oT *= R -------------------
                sflat = spool.tile([128, N], BF16, tag="sflat")
                nc.vector.tensor_copy(sflat[:], sums_ps[:])
                total_ps = opspool.tile([128, N], F32, tag="o")
                nc.tensor.matmul(
                    total_ps[0:98, :],
                    lhsT=pr_sb[:],
                    rhs=sflat[:],
                    start=True,
                    stop=True,
                )
                lg = spool.tile([98, N], F32, tag="lg")
                nc.scalar.activation(lg[:], total_ps[0:98, :], LOG_F)
                rec = spool.tile([98, N], BF16, tag="rec")
                nc.scalar.activation(rec[:], lg[:], EXP_F, scale=-1.0)
                for th in range(2):
                    rpa = psq.tile([128, N], F32, tag="ps", name=f"rpa{s}_{th}")
                    rpb = psq.tile([128, N], F32, tag="ps", name=f"rpb{s}_{th}")
                    for j, rp_ in ((0, rpa), (1, rpb)):
                        t = 2 * th + j
                        nc.tensor.matmul(
                            rp_[:],
                            lhsT=p2_sb[32 * t : 32 * t + 2, :],
                            rhs=rec[32 * t : 32 * t + 2, :],
                            start=True,
                            stop=True,
                            tile_position=(32 * t, 0),
                        )
                    for j, rp_ in ((0, rpa), (1, rpb)):
                        t = 2 * th + j
                        nc.vector.tensor_tensor(
                            oT[:, t, :], oT[:, t, :], rp_[:], MULT
                        )

                # ---- output projection ------------------------------------
                yt = ypool.tile([128, DT, N], BF16, tag="y", name=f"yts{s}")
                for dt in range(DT):
                    ps = psq.tile([128, 512], F32, tag="ps")
                    for et in range(DT):
                        nc.tensor.matmul(
                            ps[:],
                            lhsT=wo_sb[:, et, dt * 128 : (dt + 1) * 128],
                            rhs=oT[:, et, :],
                            start=(et == 0),
                            stop=(et == DT - 1),
                        )
                    nc.vector.tensor_scalar_add(
                        yt[:, dt, :], ps[:], b_sb[:, dt : dt + 1]
                    )
                nc.sync.dma_start(out[s].rearrange("t p n -> p t n"), yt[:])

    nc.compile()
    return nc


def make_in_maps(x, W_qkv, W_out, b_out):
    """Shard + lay out full inputs into the 8 per-core input maps."""
    import ml_dtypes

    b, p, n, d = x.shape
    xs = np.ascontiguousarray(x, dtype=np.float32).reshape(b * p, n, d)
    wqkv = (
        np.ascontiguousarray(W_qkv, dtype=np.float32)
        .reshape(DT, 128, 3 * D)
        .astype(ml_dtypes.bfloat16)
    )
    wout = (
        np.ascontiguousarray(W_out, dtype=np.float32)
        .reshape(DT, 128, D)
        .astype(ml_dtypes.bfloat16)
    )
    bo = np.ascontiguousarray(b_out, dtype=np.float32)

    # R-broadcast pattern: row 32t+p -> ones over cols of head parity p.
    p2 = np.zeros((128, 128), dtype=ml_dtypes.bfloat16)
    for t in range(4):
        p2[32 * t, 0:64] = 1.0
        p2[32 * t + 1, 64:128] = 1.0
    # Denominator-partial selectors: lhsT for (t, jh) has ones in col 4t+2jh.
    ones_sel = np.zeros((128, 8, 32), dtype=ml_dtypes.bfloat16)
    for t in range(4):
        for jh in range(2):
            ones_sel[:, 2 * t + jh, 4 * t + 2 * jh] = 1.0
    # Partial-reduce: row 32*(2jj+p)+4t+2jh (partial of head 2t+p over key
    # tile 2jh+jj) -> total at row 32t+p.
    pr = np.zeros((128, 98), dtype=ml_dtypes.bfloat16)
    for t in range(4):
        for pq in range(2):
            for jj in range(2):
                for jh in range(2):
                    pr[32 * (2 * jj + pq) + 4 * t + 2 * jh, 32 * t + pq] = 1.0

    in_maps = []
    for c in range(N_CORES):
        seqs = xs[c * SEQ_PER_CORE : (c + 1) * SEQ_PER_CORE]  # (4, n, d)
        xT = (
            np.ascontiguousarray(seqs.transpose(0, 2, 1))
            .reshape(SEQ_PER_CORE, DT, 128, N)
            .astype(ml_dtypes.bfloat16)
        )
        in_maps.append(
            {
                "xT": xT,
                "wqkv": wqkv,
                "wout": wout,
                "bout": bo,
                "p2d": p2,
                "onesd": ones_sel,
                "prd": pr,
            }
        )
    return in_maps


def assemble_output(results, b, p, n, d):
    """Gather per-core yT outputs back into the full (b,p,n,d) array."""
    y = np.empty((b * p, n, d), dtype=np.float32)
    for c in range(N_CORES):
        yT = np.asarray(results[c]["out"]).astype(np.float32).reshape(
            SEQ_PER_CORE, D, N
        )
        y[c * SEQ_PER_CORE : (c + 1) * SEQ_PER_CORE] = yT.transpose(0, 2, 1)
    return y.reshape(b, p, n, d)


_NC_CACHE = None


def _get_nc():
    global _NC_CACHE
    if _NC_CACHE is None:
        _NC_CACHE = build_nc()
    return _NC_CACHE


def run(inputs, trace=False, **spmd_kwargs):
    """Run on the 8 NeuronCores; returns (full_output, BassKernelResults)."""
    from concourse.bass_utils import run_bass_kernel_spmd

    x = np.asarray(inputs["x"])
    b, p, n, d = x.shape
    nc = _get_nc()
    in_maps = make_in_maps(x, inputs["W_qkv"], inputs["W_out"], inputs["b_out"])
    res = run_bass_kernel_spmd(
        nc, in_maps, core_ids=list(range(N_CORES)), trace=trace, **spmd_kwargs
    )
    return assemble_output(res.results, b, p, n, d), res


def kernel(x, W_qkv, W_out, b_out):
    out, _ = run({"x": x, "W_qkv": W_qkv, "W_out": W_out, "b_out": b_out})
    return out.astype(np.float32)


# revision 12
# speedup vs baseline: 1.3088x; 1.3088x over previous
"""Multi-head attention (b=2, p=16, n=512, d=512, h=8, dh=64) on 8 TRN2 cores.

Data-parallel over the 32 (b,p) sequences: 4 sequences per core, no
collectives.  Per-core dataflow (everything "T" = feature-on-partition):

  xT  (d,n)  --W_qkv stationary-->  qT,kT (e,n)   [e-tile = 2 heads]
  xT chunks stationary, W_v moving ->  v natural (n, h*dh)
  dots: per head pair, K=64 row-tiled (A rows 0:64 / B rows 64:128) so the
    two heads' score matmuls run CONCURRENTLY in the PE array; both heads'
    4 score banks are evacuated by ONE ScalarE exp (FD=2048), so the banks
    free atomically and the next dots pair issues back-to-back.
  attnv: per (head, key-tile), M=64 col-tiled (A cols 0:64 / B cols
    64:128) into one PSUM bank -> concurrent pair, accumulated over key
    tiles with per-element has_written semantics (single bank-clear).
  softmax denominators: tiny ones-matmuls (M=32, 4x col-tiled across the
    array) accumulate per-(head,key-tile) partials into a zeroed PSUM
    bank; one reduce matmul folds partials to per-head totals at rows
    32t/32t+1; reciprocal = ScalarE ln then exp(-x) (same table set as
    the softmax exp); R = 4x row-tiled K=2 broadcast matmuls; oT *= R.
  yT = W_out.T @ oT + b  (bf16 out, DMA to DRAM; host widens to f32)

QKV work for sequence s+1 is interleaved into the attention pairs of
sequence s so the TensorEngine never starves while ScalarE runs exp.
Host side transposes x into xT and the returned yT back to y.
"""

import os
import sys

import numpy as np

for _p in ("/opt/trn_rl_repo", "/root/.axon_site/_ro/trn_rl_repo"):
    if os.path.isdir(_p) and _p not in sys.path:
        sys.path.insert(0, _p)

import concourse.bass as bass  # noqa: E402
import concourse.mybir as mybir  # noqa: E402
from concourse import bacc  # noqa: E402
from concourse.tile import TileContext  # noqa: E402

F32 = mybir.dt.float32
BF16 = mybir.dt.bfloat16

N_CORES = 8
SEQ_PER_CORE = 4  # (b*p)=32 sequences / 8 cores
N = 512  # tokens per sequence
D = 512  # model dim
HEADS = 8
DH = 64
SCALE = DH**-0.5
NT = N // 128  # 4 token tiles
DT = D // 128  # 4 dim tiles

EXP_F = mybir.ActivationFunctionType.Exp
LOG_F = mybir.ActivationFunctionType.Ln
MULT = mybir.AluOpType.mult


def _pin_act_tables():
    """Make Exp and Ln resolve to the one table set that holds BOTH
    (natural_log_exp_and_others), so ScalarE never swaps table sets
    between the softmax exp and the reciprocal ln.  Set order (and thus
    act_func_set_id) is preserved; only the function->set resolution
    changes."""
    import concourse.hw_specs as hw_specs

    if getattr(bacc, "_act_tables_pinned", False):
        return
    real = hw_specs.get_activation_tables

    def patched(module_arch):
        tables = real(module_arch)
        both = {
            name
            for name, s in tables.items()
            if EXP_F in s and LOG_F in s
        }
        if not both:
            return tables
        out = {}
        for name, s in tables.items():
            if name not in both:
                s = s - {EXP_F, LOG_F}
            out[name] = s
        return out

    bacc.get_activation_tables = patched
    bacc._act_tables_pinned = True


def build_nc():
    """Build the per-core SPMD Bass program (same program on all 8 cores)."""
    _pin_act_tables()
    nc = bacc.Bacc("TRN2", target_bir_lowering=False)

    xT = nc.declare_dram_parameter(
        "xT", [SEQ_PER_CORE, DT, 128, N], BF16, isOutput=False
    )
    wqkv = nc.declare_dram_parameter("wqkv", [DT, 128, 3 * D], BF16, isOutput=False)
    wout = nc.declare_dram_parameter("wout", [DT, 128, D], BF16, isOutput=False)
    bout = nc.declare_dram_parameter("bout", [D], F32, isOutput=False)
    p2d = nc.declare_dram_parameter("p2d", [128, 128], BF16, isOutput=False)
    onesd = nc.declare_dram_parameter("onesd", [128, 8, 32], BF16, isOutput=False)
    prd = nc.declare_dram_parameter("prd", [128, 98], BF16, isOutput=False)
    out = nc.declare_dram_parameter(
        "out", [SEQ_PER_CORE, DT, 128, N], BF16, isOutput=True
    )

    with TileContext(nc) as tc:
        with (
            tc.tile_pool(name="consts", bufs=1) as cpool,
            tc.tile_pool(name="xin", bufs=2) as xpool,
            tc.tile_pool(name="qk", bufs=2) as qkpool,
            tc.tile_pool(name="vt", bufs=2) as vpool,
            tc.tile_pool(name="expt", bufs=2) as epool,
            tc.tile_pool(name="ot", bufs=3) as otpool,
            tc.tile_pool(name="small", bufs=2) as spool,
            tc.tile_pool(name="yout", bufs=3) as ypool,
            tc.tile_pool(name="dab", bufs=1, space="PSUM") as dabpool,
            tc.tile_pool(name="ops", bufs=1, space="PSUM") as opspool,
            tc.tile_pool(name="sums", bufs=1, space="PSUM") as smpool,
            tc.tile_pool(name="psq", bufs=2, space="PSUM") as psq,
        ):
            # ---- constants (wq Q-chunk + x first: compute starts ASAP) ----
            wq_sb = cpool.tile([128, DT, 3 * D], BF16, tag="wq")
            wo_sb = cpool.tile([128, DT, D], BF16, tag="wo")
            b_sb = cpool.tile([128, DT], F32, tag="b")
            p2_sb = cpool.tile([128, 128], BF16, tag="p2")
            ones_sb = cpool.tile([128, 8, 32], BF16, tag="ones")
            pr_sb = cpool.tile([128, 98], BF16, tag="pr")

            _WQ_CHUNKS = [(0, 128), (128, 512), (512, 1024), (1024, 1536)]

            def load_wq_chunk(c):
                lo, hi = _WQ_CHUNKS[c]
                nc.sync.dma_start(
                    wq_sb[:, :, lo:hi],
                    wqkv.rearrange("t p e -> p t e")[:, :, lo:hi],
                )

            def load_rest_consts():
                nc.sync.dma_start(wo_sb[:], wout.rearrange("t p e -> p t e"))
                nc.sync.dma_start(b_sb[:], bout.rearrange("(t p) -> p t", p=128))
                nc.sync.dma_start(p2_sb[:], p2d[:])
                nc.sync.dma_start(ones_sb[:], onesd[:])
                nc.sync.dma_start(pr_sb[:], prd[:])

            seq_tiles = {}

            def qkv_alloc(s):
                """Allocate per-seq tiles + start the xT DMA."""
                xt = xpool.tile([128, DT, N], BF16, tag="x")
                for dt in range(DT):
                    nc.sync.dma_start(
                        xt[:, dt, :], xT[s].rearrange("t p n -> p t n")[:, dt, :]
                    )
                q_sb = qkpool.tile([128, DT, N], BF16, tag="q")
                k_sb = qkpool.tile([128, DT, N], BF16, tag="k")
                vt = vpool.tile([128, NT, HEADS * DH], BF16, tag="v")
                seq_tiles[s] = (xt, q_sb, k_sb, vt)

            def qkv_etile(s, et):
                """One QKV output tile (et 0..7 = qT/kT e-tiles, 8..11 = v
                natural n-tiles): 4 accumulating matmuls + evacuation."""
                xt, q_sb, k_sb, vt = seq_tiles[s]
                ps = psq.tile([128, 512], F32, tag="ps")
                if et < 8:
                    for dt in range(DT):
                        nc.tensor.matmul(
                            ps[:],
                            lhsT=wq_sb[:, dt, et * 128 : (et + 1) * 128],
                            rhs=xt[:, dt, :],
                            start=(dt == 0),
                            stop=(dt == DT - 1),
                        )
                    dest = q_sb[:, et, :] if et < 4 else k_sb[:, et - 4, :]
                    nc.scalar.copy(dest, ps[:])
                else:
                    nt = et - 8
                    for dt in range(DT):
                        nc.tensor.matmul(
                            ps[:],
                            lhsT=xt[:, dt, nt * 128 : (nt + 1) * 128],
                            rhs=wq_sb[:, dt, 2 * D : 3 * D],
                            start=(dt == 0),
                            stop=(dt == DT - 1),
                        )
                    nc.vector.tensor_copy(vt[:, nt, :], ps[:])

            proj_state = {}

            def proj_group(s_, dt):
                """One 128-col tile of y = W_out.T @ oT + b for seq s_."""
                oT_, yt_ = proj_state[s_]
                ps = psq.tile([128, 512], F32, tag="ps")
                for et in range(DT):
                    nc.tensor.matmul(
                        ps[:],
                        lhsT=wo_sb[:, et, dt * 128 : (dt + 1) * 128],
                        rhs=oT_[:, et, :],
                        start=(et == 0),
                        stop=(et == DT - 1),
                    )
                nc.vector.tensor_scalar_add(
                    yt_[:, dt, :], ps[:], b_sb[:, dt : dt + 1]
                )
                nc.sync.dma_start(
                    out[s_].rearrange("t p n -> p t n")[:, dt, :], yt_[:, dt, :]
                )

            # ---- prologue: QKV for sequence 0 -----------------------------
            load_wq_chunk(0)
            qkv_alloc(0)
            load_wq_chunk(1)
            load_wq_chunk(2)
            load_wq_chunk(3)
            load_rest_consts()
            for et in range(12):
                qkv_etile(0, et)

            for s in range(SEQ_PER_CORE):
                _, q_sb, k_sb, vt = seq_tiles[s]
                oT = otpool.tile([128, DT, N], BF16, tag="o")
                yt = ypool.tile([128, DT, N], BF16, tag="y", name=f"yts{s}")
                proj_state[s] = (oT, yt)
                sums_ps = smpool.tile([128, N], F32, tag="sums")
                if s + 1 < SEQ_PER_CORE:
                    qkv_alloc(s + 1)

                # (seq, group) pairs of deferred output-projection work run
                # as fillers in THIS seq's pair loop; spread so the last
                # seq's pair loop still has tensor work.
                _DEFER = {
                    1: [(0, 0), (0, 1)],
                    2: [(0, 2), (0, 3), (1, 0), (1, 1)],
                    3: [(1, 2), (1, 3), (2, 0), (2, 1), (2, 2), (2, 3)],
                }
                fill = [
                    (lambda s_=ps_, d_=d: proj_group(s_, d_))
                    for (ps_, d) in _DEFER.get(s, [])
                ]
                if s + 1 < SEQ_PER_CORE:
                    fill += [(lambda s_=s + 1, e_=e: qkv_etile(s_, e_))
                             for e in range(12)]
                fi = 0

                def filler(k):
                    nonlocal fi
                    for _ in range(k):
                        if fi < len(fill):
                            fill[fi]()
                            fi += 1

                exp_tiles = {}

                def dots_half(t, jh):
                    """Scores for head pair t, key tiles 2jh/2jh+1: 4 row-
                    tiled matmuls (A/B concurrent) + ONE exp over all 4
                    banks so they free atomically."""
                    dab = dabpool.tile([128, NT, N], F32, tag="dab")
                    for jj in range(2):
                        jt = 2 * jh + jj
                        nc.tensor.matmul(
                            dab[:, jj, :],
                            lhsT=k_sb[0:64, t, jt * 128 : (jt + 1) * 128],
                            rhs=q_sb[0:64, t, :],
                            start=True,
                            stop=True,
                        )
                        nc.tensor.matmul(
                            dab[:, 2 + jj, :],
                            lhsT=k_sb[64:128, t, jt * 128 : (jt + 1) * 128],
                            rhs=q_sb[64:128, t, :],
                            start=True,
                            stop=True,
                            tile_position=(64, 0),
                        )
                    e = exp_tiles[t]
                    nc.scalar.activation(
                        e[:, jh, :, :], dab[:, :, :], EXP_F, scale=SCALE
                    )

                def attnv_half(t, jh, o_ps):
                    """attn @ v for key tiles 2jh/2jh+1: col-tiled M=64
                    pairs (A cols 0:64, B cols 64:128) into one bank."""
                    e = exp_tiles[t]
                    for jj in range(2):
                        jt = 2 * jh + jj
                        nc.tensor.matmul(
                            o_ps[0:64, :],
                            lhsT=vt[:, jt, (2 * t) * DH : (2 * t + 1) * DH],
                            rhs=e[:, jh, jj, :],
                            start=(jt == 0),
                            stop=(jt == NT - 1),
                            skip_group_check=True,
                        )
                        nc.tensor.matmul(
                            o_ps[64:128, :],
                            lhsT=vt[:, jt, (2 * t + 1) * DH : (2 * t + 2) * DH],
                            rhs=e[:, jh, 2 + jj, :],
                            start=(jt == 0),
                            stop=(jt == NT - 1),
                            skip_group_check=True,
                            tile_position=(0, 64),
                        )

                def sums_half(t, jh):
                    """Denominator partials: 4 ones-matmuls in 4 distinct
                    column groups (concurrent).  Partial for head 2t+p over
                    key tile 2jh+jj lands at row 32*(2jj+p) + 4t + 2jh."""
                    e = exp_tiles[t]
                    for jj in range(2):
                        for p in range(2):
                            g = 2 * jj + p
                            nc.tensor.matmul(
                                sums_ps[32 * g : 32 * g + 32, :],
                                lhsT=ones_sb[:, 2 * t + jh, :],
                                rhs=e[:, jh, (2 + jj) if p else jj, :],
                                start=(t == 0 and jh == 0),
                                stop=True,
                                skip_group_check=True,
                                tile_position=(0, 32 * g),
                            )

                for t in range(4):
                    exp_tiles[t] = epool.tile(
                        [128, 2, NT, N], BF16, tag="exp", name=f"exp{s}_{t}"
                    )
                    o_ps = opspool.tile([128, N], F32, tag="o")
                    dots_half(t, 0)
                    filler(1)
                    dots_half(t, 1)
                    attnv_half(t, 0, o_ps)
                    sums_half(t, 0)
                    filler(1)
                    attnv_half(t, 1, o_ps)
                    sums_half(t, 1)
                    nc.vector.tensor_copy(oT[:, t, :], o_ps[:])
                    filler(1)

                # ---- softmax denominators -> R, oT *= R -------------------
                sflat = spool.tile([128, N], BF16, tag="sflat")
                nc.vector.tensor_copy(sflat[:], sums_ps[:])
                total_ps = opspool.tile([128, N], F32, tag="o")
                nc.tensor.matmul(
                    total_ps[0:98, :],
                    lhsT=pr_sb[:],
                    rhs=sflat[:],
                    start=True,
                    stop=True,
                )
                lg = spool.tile([98, N], F32, tag="lg")
                nc.scalar.activation(lg[:], total_ps[0:98, :], LOG_F)
                rec = spool.tile([98, N], BF16, tag="rec")
                nc.scalar.activation(rec[:], lg[:], EXP_F, scale=-1.0)
                for th in range(2):
                    rpa = smpool.tile([128, N], F32, tag="sums", name=f"rpa{s}_{th}")
                    rpb = opspool.tile([128, N], F32, tag="o", name=f"rpb{s}_{th}")
                    for j, rp_ in ((0, rpa), (1, rpb)):
                        t = 2 * th + j
                        nc.tensor.matmul(
                            rp_[:],
                            lhsT=p2_sb[32 * t : 32 * t + 2, :],
                            rhs=rec[32 * t : 32 * t + 2, :],
                            start=True,
                            stop=True,
                            tile_position=(32 * t, 0),
                        )
                    for j, rp_ in ((0, rpa), (1, rpb)):
                        t = 2 * th + j
                        nc.vector.tensor_tensor(
                            oT[:, t, :], oT[:, t, :], rp_[:], MULT
                        )

                # ---- output projection ------------------------------------
                # Deferred: proj(s) runs as filler inside seq s+1's pair
                # loop (keeps the last seq's TensorE fed); drain leftovers
                # now, and run proj(s) inline for the final sequence.
                filler(len(fill))
                if s == SEQ_PER_CORE - 1:
                    for dt in range(DT):
                        proj_group(s, dt)

    nc.compile()
    return nc


def make_in_maps(x, W_qkv, W_out, b_out):
    """Shard + lay out full inputs into the 8 per-core input maps."""
    import ml_dtypes

    b, p, n, d = x.shape
    xs = np.ascontiguousarray(x, dtype=np.float32).reshape(b * p, n, d)
    wqkv = (
        np.ascontiguousarray(W_qkv, dtype=np.float32)
        .reshape(DT, 128, 3 * D)
        .astype(ml_dtypes.bfloat16)
    )
    wout = (
        np.ascontiguousarray(W_out, dtype=np.float32)
        .reshape(DT, 128, D)
        .astype(ml_dtypes.bfloat16)
    )
    bo = np.ascontiguousarray(b_out, dtype=np.float32)

    # R-broadcast pattern: row 32t+p -> ones over cols of head parity p.
    p2 = np.zeros((128, 128), dtype=ml_dtypes.bfloat16)
    for t in range(4):
        p2[32 * t, 0:64] = 1.0
        p2[32 * t + 1, 64:128] = 1.0
    # Denominator-partial selectors: lhsT for (t, jh) has ones in col 4t+2jh.
    ones_sel = np.zeros((128, 8, 32), dtype=ml_dtypes.bfloat16)
    for t in range(4):
        for jh in range(2):
            ones_sel[:, 2 * t + jh, 4 * t + 2 * jh] = 1.0
    # Partial-reduce: row 32*(2jj+p)+4t+2jh (partial of head 2t+p over key
    # tile 2jh+jj) -> total at row 32t+p.
    pr = np.zeros((128, 98), dtype=ml_dtypes.bfloat16)
    for t in range(4):
        for pq in range(2):
            for jj in range(2):
                for jh in range(2):
                    pr[32 * (2 * jj + pq) + 4 * t + 2 * jh, 32 * t + pq] = 1.0

    in_maps = []
    for c in range(N_CORES):
        seqs = xs[c * SEQ_PER_CORE : (c + 1) * SEQ_PER_CORE]  # (4, n, d)
        xT = (
            np.ascontiguousarray(seqs.transpose(0, 2, 1))
            .reshape(SEQ_PER_CORE, DT, 128, N)
            .astype(ml_dtypes.bfloat16)
        )
        in_maps.append(
            {
                "xT": xT,
                "wqkv": wqkv,
                "wout": wout,
                "bout": bo,
                "p2d": p2,
                "onesd": ones_sel,
                "prd": pr,
            }
        )
    return in_maps


def assemble_output(results, b, p, n, d):
    """Gather per-core yT outputs back into the full (b,p,n,d) array."""
    y = np.empty((b * p, n, d), dtype=np.float32)
    for c in range(N_CORES):
        yT = np.asarray(results[c]["out"]).astype(np.float32).reshape(
            SEQ_PER_CORE, D, N
        )
        y[c * SEQ_PER_CORE : (c + 1) * SEQ_PER_CORE] = yT.transpose(0, 2, 1)
    return y.reshape(b, p, n, d)


_NC_CACHE = None


def _get_nc():
    global _NC_CACHE
    if _NC_CACHE is None:
        _NC_CACHE = build_nc()
    return _NC_CACHE


def run(inputs, trace=False, **spmd_kwargs):
    """Run on the 8 NeuronCores; returns (full_output, BassKernelResults)."""
    from concourse.bass_utils import run_bass_kernel_spmd

    x = np.asarray(inputs["x"])
    b, p, n, d = x.shape
    nc = _get_nc()
    in_maps = make_in_maps(x, inputs["W_qkv"], inputs["W_out"], inputs["b_out"])
    res = run_bass_kernel_spmd(
        nc, in_maps, core_ids=list(range(N_CORES)), trace=trace, **spmd_kwargs
    )
    return assemble_output(res.results, b, p, n, d), res


def kernel(x, W_qkv, W_out, b_out):
    out, _ = run({"x": x, "W_qkv": W_qkv, "W_out": W_out, "b_out": b_out})
    return out.astype(np.float32)
